# revision 6
# baseline (speedup 1.0000x reference)
"""Causal self-attention (b=4, t=2048, c=1024, h=16, d=64) on 8 TRN2 cores.

Sharding: core i -> batch i//2, head-half i%2 (8 heads), out-col-half i%2.
Per core: QKV (bf16) for its heads over its batch, flash-style causal
attention in transposed layout with a 256-wide diagonal split (25% less
wasted work on the causal diagonal), pairwise AllGather of y^T (quartered
for the last pair so the projection streams in behind it), output
projection of this core's 512 columns. Output is exact-shape f32.
"""
import numpy as np
import ml_dtypes

import concourse.bass as bass
import concourse.mybir as mybir
import concourse.tile as tile
from concourse import bacc
from concourse.bass_utils import run_bass_kernel_spmd

BF16 = mybir.dt.bfloat16
F32 = mybir.dt.float32

B, T, C = 4, 2048, 1024
H, D = 16, 64
HL = 8            # heads per core (local)
NP = HL // 2      # head pairs per core
QB = 512          # q block (free dim)
NQB = T // QB     # 4 q blocks
NKT = T // 128    # 16 kv tiles
NCH = C // 128    # 8 contraction chunks

_NC_CACHE = {}


def build_nc():
    if "nc" in _NC_CACHE:
        return _NC_CACHE["nc"]
    nc = bacc.Bacc("TRN2", target_bir_lowering=False, debug=False, num_devices=8)
    xT = nc.dram_tensor("xT", [C, T], BF16, kind="ExternalInput")
    wk = nc.dram_tensor("wk", [C, 512], BF16, kind="ExternalInput")
    wq = nc.dram_tensor("wq", [C, 512], BF16, kind="ExternalInput")
    wv = nc.dram_tensor("wv", [C, 512], BF16, kind="ExternalInput")
    wp = nc.dram_tensor("wp", [C, 512], BF16, kind="ExternalInput")
    msk2 = nc.dram_tensor("msk2", [128, 512], BF16, kind="ExternalInput")
    out = nc.dram_tensor("out", [T, 512], F32, kind="ExternalOutput")

    with tile.TileContext(nc) as tc:
        with tc.tile_pool(name="w", bufs=1) as wpool, \
             tc.tile_pool(name="x", bufs=1) as xpool, \
             tc.tile_pool(name="kqv", bufs=1) as kqv, \
             tc.tile_pool(name="att", bufs=2) as att, \
             tc.tile_pool(name="y", bufs=1) as ypool, \
             tc.tile_pool(name="ps", bufs=1, space="PSUM") as ps, \
             tc.tile_pool(name="dram", bufs=1, space="DRAM") as dram:

            # ---- input loads, ordered by first use ----
            xT_sb = [xpool.tile([128, T], BF16, name=f"xT{ch}") for ch in range(NCH)]
            wk_sb = [wpool.tile([128, 512], BF16, name=f"wk{ch}") for ch in range(NCH)]
            wq_sb = [wpool.tile([128, 512], BF16, name=f"wq{ch}") for ch in range(NCH)]
            wv_sb = [wpool.tile([128, 512], BF16, name=f"wv{ch}") for ch in range(NCH)]
            wp_sb = [wpool.tile([128, 512], BF16, name=f"wp{ch}") for ch in range(NCH)]
            msk2_sb = wpool.tile([128, 512], BF16)

            def eng_for(i):
                return nc.sync if i % 2 == 0 else nc.scalar

            nc.scalar.dma_start(out=msk2_sb[:], in_=msk2.ap()[:])
            for ch in range(NCH):
                sl = slice(128 * ch, 128 * (ch + 1))
                eng_for(ch).dma_start(out=wk_sb[ch][:], in_=wk.ap()[sl, :])
                eng_for(ch + 1).dma_start(out=xT_sb[ch][:, 0:512], in_=xT.ap()[sl, 0:512])
            for ch in range(NCH):
                sl = slice(128 * ch, 128 * (ch + 1))
                eng_for(ch).dma_start(out=wq_sb[ch][:], in_=wq.ap()[sl, :])
            for ch in range(NCH):
                sl = slice(128 * ch, 128 * (ch + 1))
                eng_for(ch).dma_start(out=wv_sb[ch][:], in_=wv.ap()[sl, :])
            for blk in range(1, 4):
                for ch in range(NCH):
                    sl = slice(128 * ch, 128 * (ch + 1))
                    eng_for(ch).dma_start(out=xT_sb[ch][:, 512 * blk:512 * (blk + 1)],
                                          in_=xT.ap()[sl, 512 * blk:512 * (blk + 1)])
            for ch in range(NCH):
                nc.scalar.dma_start(out=wp_sb[ch][:], in_=wp.ap()[128 * ch:128 * (ch + 1), :])

            # ---- V (token-major, with ones column per head) ----
            v_sb = [kqv.tile([128, HL * 65], BF16, name=f"v{tt}") for tt in range(NKT)]

            def v_group(tt):
                v3 = v_sb[tt].rearrange("p (g e) -> p g e", g=HL)
                v_ps = ps.tile([128, 512], F32, tag="qkv", bufs=2, name="vps")
                for ch in range(NCH):
                    nc.tensor.matmul(
                        v_ps[:],
                        xT_sb[ch][:, 128 * tt:128 * (tt + 1)],
                        wv_sb[ch][:],
                        start=(ch == 0), stop=(ch == NCH - 1),
                    )
                nc.vector.tensor_copy(
                    out=v3[:, :, 0:64],
                    in_=v_ps[:].rearrange("p (g e) -> p g e", g=HL),
                )
                nc.vector.memset(v3[:, :, 64:65], 1.0)

            # ---- K^T, Q^T (head-pair tiles [128, T]) ----
            kT_sb = [kqv.tile([128, T], BF16, name=f"kT{p}") for p in range(NP)]
            qT_sb = [kqv.tile([128, T], BF16, name=f"qT{p}") for p in range(NP)]
            yT_sb = [ypool.tile([128, T], BF16, name=f"yT{p}", tag="yt", bufs=NP) for p in range(NP)]
            sums_pack = ypool.tile([128, 128], F32)
            recip_pack = ypool.tile([128, 128], F32)
            recip_pbf = ypool.tile([128, 128], BF16)

            rdram = dram.tile([HL, T], BF16)
            cc_in = [dram.tile([128, T], BF16, name=f"ccin{p}") for p in range(NP - 1)]
            cc_out = [dram.tile([256, T], BF16, name=f"ccout{p}") for p in range(NP - 1)]
            cc_in4 = [dram.tile([128, 512], BF16, name=f"ccin4q{q}") for q in range(NQB)]
            cc_out4 = [dram.tile([256, 512], BF16, name=f"ccout4q{q}") for q in range(NQB)]
            yg_sb = [ypool.tile([128, T], BF16, name=f"yg{j}", tag="yg", bufs=2 * NP) for j in range(2 * NP)]
            o_part = [ypool.tile([128, 512], BF16, name=f"opart{i}", tag="opart", bufs=4)
                      for i in range(4)]

            GROUPS2 = [[0, 1], [2, 3], [4, 5], [6, 7]]

            def kq_group(p, w_sb, dst, nb, evac_eng=None):
                kq_ps = ps.tile([128, 512], F32, tag="qkv", bufs=2, name="kqps")
                for ch in range(NCH):
                    nc.tensor.matmul(
                        kq_ps[:],
                        w_sb[ch][:, 128 * p:128 * (p + 1)],
                        xT_sb[ch][:, 512 * nb:512 * (nb + 1)],
                        start=(ch == 0), stop=(ch == NCH - 1),
                    )
                if evac_eng == "scalar":
                    nc.scalar.copy(out=dst[p][:, 512 * nb:512 * (nb + 1)], in_=kq_ps[:])
                else:
                    nc.vector.tensor_copy(out=dst[p][:, 512 * nb:512 * (nb + 1)], in_=kq_ps[:])

            def kq_thunks_list(p):
                return [
                    (lambda w_sb=w_sb, dst=dst, nb=nb: kq_group(p, w_sb, dst, nb))
                    for (w_sb, dst) in ((wk_sb, kT_sb), (wq_sb, qT_sb))
                    for nb in range(NQB)
                ]

            # ---- projection helpers ----
            chunksA = [(r, pp) for r in range(2) for pp in range(NP - 1)]
            chunksFull = [(0, NP - 1), (1, NP - 1)] + chunksA

            def proj_full(tt):
                o_ps = ps.tile([128, 512], F32, tag="qkv", bufs=2, name="ops")
                for ci, (r, pp) in enumerate(chunksFull):
                    nc.tensor.matmul(
                        o_ps[:],
                        yg_sb[r * NP + pp][:, 128 * tt:128 * (tt + 1)],
                        wp_sb[4 * r + pp][:],
                        start=(ci == 0), stop=(ci == len(chunksFull) - 1),
                    )
                o_sbf = att.tile([128, 512], F32, tag="osb", bufs=3)
                nc.vector.tensor_copy(out=o_sbf[:], in_=o_ps[:])
                nc.sync.dma_start(out=out.ap()[128 * tt:128 * (tt + 1), :], in_=o_sbf[:])

            def proj_partial(tt):
                o_ps = ps.tile([128, 512], F32, tag="qkv", bufs=2, name="ops")
                for ci, (r, pp) in enumerate(chunksA):
                    nc.tensor.matmul(
                        o_ps[:],
                        yg_sb[r * NP + pp][:, 128 * tt:128 * (tt + 1)],
                        wp_sb[4 * r + pp][:],
                        start=(ci == 0), stop=(ci == len(chunksA) - 1),
                    )
                nc.vector.tensor_copy(out=o_part[tt - 12][:], in_=o_ps[:])

            # ---- upfront pair-0 QKV (just what attention qb0 needs) ----
            kq_group(0, wk_sb, kT_sb, 0, "scalar")
            kq_group(0, wq_sb, qT_sb, 0, "scalar")
            for tt in range(4):
                v_group(tt)

            def make_pend(p):
                if p == 0:
                    items = []
                    for nb in range(1, NQB):
                        items.append((("kq0", nb),
                                      lambda nb=nb: kq_group(0, wk_sb, kT_sb, nb)))
                        items.append((("kq0", nb),
                                      lambda nb=nb: kq_group(0, wq_sb, qT_sb, nb)))
                        for tt in range(4 * nb, 4 * nb + 4):
                            items.append((("v", tt), lambda tt=tt: v_group(tt)))
                    items += [(None, th) for th in kq_thunks_list(1)]
                    return items
                if p < NP - 1:
                    return [(None, th) for th in kq_thunks_list(p + 1)]
                return []

            def ensure(pend, need_nb, need_tt):
                def blocked():
                    for key, _ in pend:
                        if key is None:
                            continue
                        if key[0] == "kq0" and key[1] <= need_nb:
                            return True
                        if key[0] == "v" and key[1] <= need_tt:
                            return True
                    return False
                while pend and blocked():
                    pend.pop(0)[1]()

            # ---- attention ----
            for p in range(NP):
                pend = make_pend(p)
                pend_late = []
                if p == NP - 2:
                    # pair-0 gather is long done; pull it into SBUF now
                    nc.sync.dma_start(out=yg_sb[0][:], in_=cc_out[0][0:128, :])
                    nc.sync.dma_start(out=yg_sb[NP][:], in_=cc_out[0][128:256, :])
                if p == NP - 1:
                    nc.sync.dma_start(out=yg_sb[1][:], in_=cc_out[1][0:128, :])
                    nc.sync.dma_start(out=yg_sb[NP + 1][:], in_=cc_out[1][128:256, :])

                    def yg_pair2():
                        nc.sync.dma_start(out=yg_sb[2][:], in_=cc_out[2][0:128, :])
                        nc.sync.dma_start(out=yg_sb[NP + 2][:], in_=cc_out[2][128:256, :])
                    pend.append((None, yg_pair2))

                for qb in range(NQB):
                    base = 4 * qb
                    y_ps = [ps.tile([128, 512], F32, tag=f"y{h}", bufs=1, name=f"yps{h}")
                            for h in range(2)]
                    # (kind, sq, first kv tile, masked)
                    groups = [("d", 0, base + 0, True),
                              ("d", 1, base + 0, False),
                              ("d", 1, base + 2, True)]
                    groups += [("o", None, kt, False) for kt in range(base)]
                    ngrp = len(groups)
                    for gi, (kind, sq, k0, masked) in enumerate(groups):
                        if p == 0:
                            need_tt = (k0 + 1) if kind == "d" else k0
                            ensure(pend, qb, need_tt)
                        s_ps = ps.tile([128, 1024], F32, tag="s", bufs=2)
                        if kind == "d":
                            q0 = QB * qb + 256 * sq
                            for h in range(2):
                                hsl = slice(64 * h, 64 * (h + 1))
                                for ji in range(2):
                                    kt = k0 + ji
                                    c0 = 256 * (2 * h + ji)
                                    # one start per PSUM bank: the bank-wide
                                    # has_written clear covers the sibling
                                    # 256-chunk, which then overwrites
                                    nc.tensor.matmul(
                                        s_ps[:, c0:c0 + 256],
                                        kT_sb[p][hsl, 128 * kt:128 * (kt + 1)],
                                        qT_sb[p][hsl, q0:q0 + 256],
                                        start=(ji == 0), stop=(ji == 1),
                                    )
                        else:
                            kt = k0
                            for h in range(2):
                                hsl = slice(64 * h, 64 * (h + 1))
                                nc.tensor.matmul(
                                    s_ps[:, 512 * h:512 * (h + 1)],
                                    kT_sb[p][hsl, 128 * kt:128 * (kt + 1)],
                                    qT_sb[p][hsl, QB * qb:QB * (qb + 1)],
                                    start=True, stop=True,
                                )
                        p_sb = att.tile([128, 1024], BF16, tag="p", bufs=4)
                        nc.scalar.activation(
                            out=p_sb[:],
                            in_=s_ps[:],
                            func=mybir.ActivationFunctionType.Exp,
                            scale=float(D) ** -0.5,
                        )
                        if masked:
                            for h in range(2):
                                nc.vector.tensor_mul(
                                    out=p_sb[:, 512 * h:512 * (h + 1)],
                                    in0=p_sb[:, 512 * h:512 * (h + 1)],
                                    in1=msk2_sb[:],
                                )
                        if kind == "d":
                            for h in range(2):
                                v3s = [v_sb[k0 + ji].rearrange("p (g e) -> p g e", g=HL)
                                       for ji in range(2)]
                                for ji in range(2):
                                    c0 = 256 * (2 * h + ji)
                                    # y_ps is one PSUM bank: only the very
                                    # first write of the q block starts it
                                    nc.tensor.matmul(
                                        y_ps[h][0:65, 256 * sq:256 * (sq + 1)],
                                        v3s[ji][:, 2 * p + h, :],
                                        p_sb[:, c0:c0 + 256],
                                        start=(sq == 0 and masked and ji == 0),
                                        stop=(qb == 0 and sq == 1 and masked and ji == 1),
                                    )
                        else:
                            for h in range(2):
                                v3 = v_sb[kt].rearrange("p (g e) -> p g e", g=HL)
                                nc.tensor.matmul(
                                    y_ps[h][0:65, :],
                                    v3[:, 2 * p + h, :],
                                    p_sb[:, 512 * h:512 * (h + 1)],
                                    start=False, stop=(kt == base - 1),
                                )
                        if pend:
                            if p == 0:
                                if gi >= 1:
                                    pend.pop(0)[1]()
                                    if len(pend) > 12 and pend[0][0] is not None:
                                        pend.pop(0)[1]()
                            elif p == NP - 1:
                                if gi >= 1:
                                    pend.pop(0)[1]()
                            elif qb >= 1 and (gi % 2 == 1 or len(pend) > (NQB - qb) * 4):
                                pend.pop(0)[1]()
                        if pend_late and ((qb == 2 and gi >= 1) or (qb == 3 and gi >= 5)):
                            pend_late.pop(0)()

                    # ---- evacuate y (unnormalized) + packed sums ----
                    qsl = slice(QB * qb, QB * (qb + 1))
                    nc.vector.tensor_copy(out=yT_sb[p][0:64, qsl], in_=y_ps[0][0:64, :])
                    ytmp = att.tile([64, 512], BF16, tag="ytmp", bufs=2)
                    nc.vector.tensor_copy(out=ytmp[:], in_=y_ps[1][0:64, :])
                    nc.sync.dma_start(out=yT_sb[p][64:128, qsl], in_=ytmp[:])
                    for h in range(2):
                        stg = att.tile([65, 512], F32, tag=f"sumstg{h}", bufs=2, name=f"stg{h}")
                        nc.vector.tensor_copy(out=stg[64:65, :], in_=y_ps[h][64:65, :])
                        u = 8 * p + 2 * qb + h
                        nc.sync.dma_start(out=sums_pack[4 * u:4 * (u + 1), :], in_=stg[64:65, :])
                    # per-qb normalize (recip over the pair's 32 rows is idempotent)
                    prows = slice(32 * p, 32 * (p + 1))
                    nc.vector.reciprocal(out=recip_pack[prows, :], in_=sums_pack[prows, :])
                    nc.vector.tensor_copy(out=recip_pbf[prows, :], in_=recip_pack[prows, :])
                    for h in range(2):
                        bse = 32 * p + 8 * qb + 4 * h
                        nc.sync.dma_start(
                            out=rdram[2 * p + h:2 * p + h + 1, qsl],
                            in_=recip_pbf[bse:bse + 4, :],
                        )
                    rb = att.tile([128, QB], BF16, tag="rb", bufs=2)
                    nc.sync.dma_start(out=rb[0:64, :], in_=rdram[2 * p:2 * p + 1, qsl].to_broadcast([64, QB]))
                    nc.sync.dma_start(out=rb[64:128, :], in_=rdram[2 * p + 1:2 * p + 2, qsl].to_broadcast([64, QB]))
                    nc.vector.tensor_mul(out=yT_sb[p][:, qsl], in0=yT_sb[p][:, qsl], in1=rb[:])

                    if p == NP - 1:
                        # quarter AllGather of this q block's y^T
                        nc.sync.dma_start(out=cc_in4[qb][:], in_=yT_sb[p][:, qsl])
                        nc.gpsimd.collective_compute(
                            "AllGather",
                            mybir.AluOpType.bypass,
                            replica_groups=GROUPS2,
                            ins=[cc_in4[qb][:].opt()],
                            outs=[cc_out4[qb][:].opt()],
                        )
                        if qb < NQB - 1:
                            def yg_q(qb=qb):
                                qs = slice(QB * qb, QB * (qb + 1))
                                nc.sync.dma_start(out=yg_sb[NP - 1][:, qs], in_=cc_out4[qb][0:128, :])
                                nc.sync.dma_start(out=yg_sb[2 * NP - 1][:, qs], in_=cc_out4[qb][128:256, :])
                            pend_late.append(yg_q)
                            for tt in range(4 * qb, 4 * qb + 4):
                                pend_late.append(lambda tt=tt: proj_full(tt))
                            if qb == 2:
                                # partials only need pairs 0-2: fine anywhere in qb3
                                for tt in range(12, 16):
                                    pend.append((None, lambda tt=tt: proj_partial(tt)))

                while pend:
                    pend.pop(0)[1]()
                while pend_late:
                    pend_late.pop(0)()
                if p < NP - 1:
                    nc.sync.dma_start(out=cc_in[p][:], in_=yT_sb[p][:])
                    nc.gpsimd.collective_compute(
                        "AllGather",
                        mybir.AluOpType.bypass,
                        replica_groups=GROUPS2,
                        ins=[cc_in[p][:].opt()],
                        outs=[cc_out[p][:].opt()],
                    )

            # ---- tail: last quarter arrives, finish tts 12..15 ----
            qs3 = slice(QB * 3, QB * 4)
            nc.sync.dma_start(out=yg_sb[NP - 1][:, qs3], in_=cc_out4[3][0:128, :])
            nc.scalar.dma_start(out=yg_sb[2 * NP - 1][:, qs3], in_=cc_out4[3][128:256, :])
            for tt in range(12, 16):
                o_ps2 = ps.tile([128, 512], F32, tag="qkv", bufs=2)
                for ri in range(2):
                    nc.tensor.matmul(
                        o_ps2[:],
                        yg_sb[ri * NP + NP - 1][:, 128 * tt:128 * (tt + 1)],
                        wp_sb[4 * ri + NP - 1][:],
                        start=(ri == 0), stop=(ri == 1),
                    )
                o_sb = att.tile([128, 512], F32, tag="osb", bufs=3)
                nc.vector.scalar_tensor_tensor(
                    out=o_sb[:], in0=o_ps2[:], scalar=1.0, in1=o_part[tt - 12][:],
                    op0=mybir.AluOpType.mult, op1=mybir.AluOpType.add,
                )
                eng = nc.sync if tt % 2 == 0 else nc.scalar
                eng.dma_start(out=out.ap()[128 * tt:128 * (tt + 1), :], in_=o_sb[:])

    nc.compile()
    _NC_CACHE["nc"] = nc
    return nc


def make_in_maps(x, w_qkv, w_proj):
    bf = ml_dtypes.bfloat16
    # 256-wide causal staircase masks for the two diagonal kv-tile offsets
    i = np.arange(128)[:, None]
    j = np.arange(256)[None, :]
    msk2 = np.concatenate(
        [(i <= j).astype(bf), (128 + i <= j).astype(bf)], axis=1
    )  # [128, 512]
    in_maps = []
    for core in range(8):
        beta, eta = core // 2, core % 2
        xT = np.ascontiguousarray(x[beta].T).astype(bf)
        wk = w_qkv[:, C + 512 * eta: C + 512 * (eta + 1)].astype(bf)
        wq = w_qkv[:, 512 * eta: 512 * (eta + 1)].astype(bf)
        wv = w_qkv[:, 2 * C + 512 * eta: 2 * C + 512 * (eta + 1)].astype(bf)
        wp = w_proj[:, 512 * eta: 512 * (eta + 1)].astype(bf)
        in_maps.append({"xT": xT, "wk": np.ascontiguousarray(wk),
                        "wq": np.ascontiguousarray(wq),
                        "wv": np.ascontiguousarray(wv),
                        "wp": np.ascontiguousarray(wp), "msk2": msk2})
    return in_maps


def assemble(results):
    out = np.empty((B, T, C), np.float32)
    for core in range(8):
        beta, eta = core // 2, core % 2
        out[beta, :, 512 * eta: 512 * (eta + 1)] = results[core]["out"]
    return out


def kernel(x, w_qkv, w_proj):
    x = np.asarray(x, np.float32)
    w_qkv = np.asarray(w_qkv, np.float32)
    w_proj = np.asarray(w_proj, np.float32)
    nc = build_nc()
    in_maps = make_in_maps(x, w_qkv, w_proj)
    res = run_bass_kernel_spmd(nc, in_maps, core_ids=list(range(8)))
    return assemble(res.results)


# revision 13
# speedup vs baseline: 1.0388x; 1.0388x over previous
"""Causal self-attention (b=4, t=2048, c=1024, h=16, d=64) on 8 TRN2 cores.

Sharding: core i -> batch i//2, head-half i%2 (8 heads), out-col-half i%2.
Per core: QKV (bf16) for its heads over its batch, flash-style causal
attention in transposed layout with a 256-wide diagonal split (25% less
wasted work on the causal diagonal), pairwise AllGather of y^T (quartered
for the last pair so the projection streams in behind it), output
projection of this core's 512 columns. Output is exact-shape f32.
"""
import numpy as np
import ml_dtypes

import concourse.bass as bass
import concourse.mybir as mybir
import concourse.tile as tile
from concourse import bacc
from concourse.bass_utils import run_bass_kernel_spmd

BF16 = mybir.dt.bfloat16
F32 = mybir.dt.float32

B, T, C = 4, 2048, 1024
H, D = 16, 64
HL = 8            # heads per core (local)
NP = HL // 2      # head pairs per core
QB = 512          # q block (free dim)
NQB = T // QB     # 4 q blocks
NKT = T // 128    # 16 kv tiles
NCH = C // 128    # 8 contraction chunks

_NC_CACHE = {}


def build_nc():
    if "nc" in _NC_CACHE:
        return _NC_CACHE["nc"]
    nc = bacc.Bacc("TRN2", target_bir_lowering=False, debug=False, num_devices=8)
    xT = nc.dram_tensor("xT", [C, T], BF16, kind="ExternalInput")
    wkqv = nc.dram_tensor("wkqv", [C, 1536], BF16, kind="ExternalInput")
    wp = nc.dram_tensor("wp", [C, 512], BF16, kind="ExternalInput")
    msk2 = nc.dram_tensor("msk2", [128, 512], BF16, kind="ExternalInput")
    out = nc.dram_tensor("out", [T, 512], F32, kind="ExternalOutput")

    with tile.TileContext(nc) as tc:
        with tc.tile_pool(name="w", bufs=1) as wpool, \
             tc.tile_pool(name="x", bufs=1) as xpool, \
             tc.tile_pool(name="kqv", bufs=1) as kqv, \
             tc.tile_pool(name="att", bufs=2) as att, \
             tc.tile_pool(name="y", bufs=1) as ypool, \
             tc.tile_pool(name="ps", bufs=1, space="PSUM") as ps, \
             tc.tile_pool(name="dram", bufs=1, space="DRAM") as dram:

            # ---- input loads, ordered by first use, 3KB descriptor rows ----
            xT_sb = [xpool.tile([128, T], BF16, name=f"xT{ch}") for ch in range(NCH)]
            wkqv_sb = [wpool.tile([128, 1536], BF16, name=f"wkqv{ch}") for ch in range(NCH)]
            WK, WQ, WV = 0, 512, 1024   # column offsets in wkqv_sb
            wp_sb = [wpool.tile([128, 512], BF16, name=f"wp{ch}") for ch in range(NCH)]
            msk2_sb = wpool.tile([128, 512], BF16)

            def eng_for(i):
                return nc.sync if i % 2 == 0 else nc.scalar

            nc.scalar.dma_start(out=msk2_sb[:], in_=msk2.ap()[:])
            for ch in range(NCH):
                sl = slice(128 * ch, 128 * (ch + 1))
                eng_for(ch).dma_start(out=wkqv_sb[ch][:], in_=wkqv.ap()[sl, :])
                eng_for(ch + 1).dma_start(out=xT_sb[ch][:, 0:512], in_=xT.ap()[sl, 0:512])
            for ch in range(NCH):
                sl = slice(128 * ch, 128 * (ch + 1))
                eng_for(ch).dma_start(out=xT_sb[ch][:, 512:T], in_=xT.ap()[sl, 512:T])
            for ch in range(NCH):
                nc.scalar.dma_start(out=wp_sb[ch][:], in_=wp.ap()[128 * ch:128 * (ch + 1), :])

            # ---- V (token-major, with ones column per head) ----
            v_sb = [kqv.tile([128, HL * 65], BF16, name=f"v{tt}") for tt in range(NKT)]

            def v_group(tt):
                v3 = v_sb[tt].rearrange("p (g e) -> p g e", g=HL)
                v_ps = ps.tile([128, 512], F32, tag="qkv", bufs=2, name="vps")
                for ch in range(NCH):
                    nc.tensor.matmul(
                        v_ps[:],
                        xT_sb[ch][:, 128 * tt:128 * (tt + 1)],
                        wkqv_sb[ch][:, WV:WV + 512],
                        start=(ch == 0), stop=(ch == NCH - 1),
                    )
                nc.vector.tensor_copy(
                    out=v3[:, :, 0:64],
                    in_=v_ps[:].rearrange("p (g e) -> p g e", g=HL),
                )
                nc.vector.memset(v3[:, :, 64:65], 1.0)

            # ---- K^T, Q^T (head-pair tiles [128, T]) ----
            kT_sb = [kqv.tile([128, T], BF16, name=f"kT{p}") for p in range(NP)]
            qT_sb = [kqv.tile([128, T], BF16, name=f"qT{p}") for p in range(NP)]
            yT_sb = [ypool.tile([128, T], BF16, name=f"yT{p}", tag="yt", bufs=NP) for p in range(NP)]
            sums_pack = ypool.tile([128, 128], F32)
            recip_pack = ypool.tile([128, 128], F32)
            recip_pbf = ypool.tile([128, 128], BF16)

            rdram = dram.tile([HL, T], BF16)
            cc_in = [dram.tile([128, T], BF16, name=f"ccin{p}") for p in range(NP - 1)]
            cc_out = [dram.tile([256, T], BF16, name=f"ccout{p}") for p in range(NP - 1)]
            cc_in4 = [dram.tile([128, 512], BF16, name=f"ccin4q{q}") for q in range(NQB)]
            cc_out4 = [dram.tile([256, 512], BF16, name=f"ccout4q{q}") for q in range(NQB)]
            yg_sb = [ypool.tile([128, T], BF16, name=f"yg{j}", tag="yg", bufs=2 * NP) for j in range(2 * NP)]
            o_part = [ypool.tile([128, 512], BF16, name=f"opart{i}", tag="opart", bufs=4)
                      for i in range(4)]

            GROUPS2 = [[0, 1], [2, 3], [4, 5], [6, 7]]

            def kq_group(p, woff, dst, nb, evac_eng=None):
                kq_ps = ps.tile([128, 512], F32, tag="qkv", bufs=2, name="kqps")
                for ch in range(NCH):
                    nc.tensor.matmul(
                        kq_ps[:],
                        wkqv_sb[ch][:, woff + 128 * p:woff + 128 * (p + 1)],
                        xT_sb[ch][:, 512 * nb:512 * (nb + 1)],
                        start=(ch == 0), stop=(ch == NCH - 1),
                    )
                if evac_eng == "scalar":
                    nc.scalar.copy(out=dst[p][:, 512 * nb:512 * (nb + 1)], in_=kq_ps[:])
                else:
                    nc.vector.tensor_copy(out=dst[p][:, 512 * nb:512 * (nb + 1)], in_=kq_ps[:])

            def kq_thunks_list(p):
                return [
                    (lambda woff=woff, dst=dst, nb=nb: kq_group(p, woff, dst, nb))
                    for (woff, dst) in ((WK, kT_sb), (WQ, qT_sb))
                    for nb in range(NQB)
                ]

            # ---- projection helpers ----
            chunksA = [(r, pp) for r in range(2) for pp in range(NP - 1)]
            chunksFull = [(0, NP - 1), (1, NP - 1)] + chunksA

            def proj_full(tt):
                o_ps = ps.tile([128, 512], F32, tag="qkv", bufs=2, name="ops")
                for ci, (r, pp) in enumerate(chunksFull):
                    nc.tensor.matmul(
                        o_ps[:],
                        yg_sb[r * NP + pp][:, 128 * tt:128 * (tt + 1)],
                        wp_sb[4 * r + pp][:],
                        start=(ci == 0), stop=(ci == len(chunksFull) - 1),
                    )
                o_sbf = att.tile([128, 512], F32, tag="osb", bufs=3)
                nc.vector.tensor_copy(out=o_sbf[:], in_=o_ps[:])
                nc.sync.dma_start(out=out.ap()[128 * tt:128 * (tt + 1), :], in_=o_sbf[:])

            def proj_partial(tt):
                o_ps = ps.tile([128, 512], F32, tag="qkv", bufs=2, name="ops")
                for ci, (r, pp) in enumerate(chunksA):
                    nc.tensor.matmul(
                        o_ps[:],
                        yg_sb[r * NP + pp][:, 128 * tt:128 * (tt + 1)],
                        wp_sb[4 * r + pp][:],
                        start=(ci == 0), stop=(ci == len(chunksA) - 1),
                    )
                nc.vector.tensor_copy(out=o_part[tt - 12][:], in_=o_ps[:])

            # ---- upfront pair-0 QKV (just what attention qb0 needs) ----
            kq_group(0, WK, kT_sb, 0, "scalar")
            kq_group(0, WQ, qT_sb, 0, "scalar")
            for tt in range(4):
                v_group(tt)

            def make_pend(p):
                if p == 0:
                    items = []
                    for nb in range(1, NQB):
                        items.append((("kq0", nb),
                                      lambda nb=nb: kq_group(0, WK, kT_sb, nb)))
                        items.append((("kq0", nb),
                                      lambda nb=nb: kq_group(0, WQ, qT_sb, nb)))
                        for tt in range(4 * nb, 4 * nb + 4):
                            items.append((("v", tt), lambda tt=tt: v_group(tt)))
                    items += [(None, th) for th in kq_thunks_list(1)]
                    return items
                if p < NP - 1:
                    return [(None, th) for th in kq_thunks_list(p + 1)]
                return []

            def ensure(pend, need_nb, need_tt):
                def blocked():
                    for key, _ in pend:
                        if key is None:
                            continue
                        if key[0] == "kq0" and key[1] <= need_nb:
                            return True
                        if key[0] == "v" and key[1] <= need_tt:
                            return True
                    return False
                while pend and blocked():
                    pend.pop(0)[1]()

            # ---- attention ----
            for p in range(NP):
                pend = make_pend(p)
                pend_late = []
                if p == NP - 2:
                    # pair-0 gather is long done; pull it into SBUF now
                    nc.sync.dma_start(out=yg_sb[0][:], in_=cc_out[0][0:128, :])
                    nc.sync.dma_start(out=yg_sb[NP][:], in_=cc_out[0][128:256, :])
                if p == NP - 1:
                    nc.sync.dma_start(out=yg_sb[1][:], in_=cc_out[1][0:128, :])
                    nc.sync.dma_start(out=yg_sb[NP + 1][:], in_=cc_out[1][128:256, :])

                    def yg_pair2():
                        nc.sync.dma_start(out=yg_sb[2][:], in_=cc_out[2][0:128, :])
                        nc.sync.dma_start(out=yg_sb[NP + 2][:], in_=cc_out[2][128:256, :])
                    pend.append((None, yg_pair2))

                for qb in range(NQB):
                    base = 4 * qb
                    y_ps = [ps.tile([128, 512], F32, tag=f"y{h}", bufs=1, name=f"yps{h}")
                            for h in range(2)]
                    # (kind, sq, first kv tile, masked)
                    groups = [("d", 0, base + 0, True),
                              ("d", 1, base + 0, False),
                              ("d", 1, base + 2, True)]
                    groups += [("o", None, kt, False) for kt in range(base)]
                    ngrp = len(groups)
                    for gi, (kind, sq, k0, masked) in enumerate(groups):
                        if p == 0:
                            need_tt = (k0 + 1) if kind == "d" else k0
                            ensure(pend, qb, need_tt)
                        s_ps = ps.tile([128, 1024], F32, tag="s", bufs=2)
                        if kind == "d":
                            q0 = QB * qb + 256 * sq
                            for h in range(2):
                                hsl = slice(64 * h, 64 * (h + 1))
                                for ji in range(2):
                                    kt = k0 + ji
                                    c0 = 256 * (2 * h + ji)
                                    # one start per PSUM bank: the bank-wide
                                    # has_written clear covers the sibling
                                    # 256-chunk, which then overwrites
                                    nc.tensor.matmul(
                                        s_ps[:, c0:c0 + 256],
                                        kT_sb[p][hsl, 128 * kt:128 * (kt + 1)],
                                        qT_sb[p][hsl, q0:q0 + 256],
                                        start=(ji == 0), stop=(ji == 1),
                                    )
                        else:
                            kt = k0
                            for h in range(2):
                                hsl = slice(64 * h, 64 * (h + 1))
                                nc.tensor.matmul(
                                    s_ps[:, 512 * h:512 * (h + 1)],
                                    kT_sb[p][hsl, 128 * kt:128 * (kt + 1)],
                                    qT_sb[p][hsl, QB * qb:QB * (qb + 1)],
                                    start=True, stop=True,
                                )
                        p_sb = att.tile([128, 1024], BF16, tag="p", bufs=4)
                        nc.scalar.activation(
                            out=p_sb[:],
                            in_=s_ps[:],
                            func=mybir.ActivationFunctionType.Exp,
                            scale=float(D) ** -0.5,
                        )
                        if masked:
                            for h in range(2):
                                nc.vector.tensor_mul(
                                    out=p_sb[:, 512 * h:512 * (h + 1)],
                                    in0=p_sb[:, 512 * h:512 * (h + 1)],
                                    in1=msk2_sb[:],
                                )
                        if kind == "d":
                            for h in range(2):
                                v3s = [v_sb[k0 + ji].rearrange("p (g e) -> p g e", g=HL)
                                       for ji in range(2)]
                                for ji in range(2):
                                    c0 = 256 * (2 * h + ji)
                                    # y_ps is one PSUM bank: only the very
                                    # first write of the q block starts it
                                    nc.tensor.matmul(
                                        y_ps[h][0:65, 256 * sq:256 * (sq + 1)],
                                        v3s[ji][:, 2 * p + h, :],
                                        p_sb[:, c0:c0 + 256],
                                        start=(sq == 0 and masked and ji == 0),
                                        stop=(qb == 0 and sq == 1 and masked and ji == 1),
                                    )
                        else:
                            for h in range(2):
                                v3 = v_sb[kt].rearrange("p (g e) -> p g e", g=HL)
                                nc.tensor.matmul(
                                    y_ps[h][0:65, :],
                                    v3[:, 2 * p + h, :],
                                    p_sb[:, 512 * h:512 * (h + 1)],
                                    start=False, stop=(kt == base - 1),
                                )
                        if pend:
                            if p == 0:
                                if gi >= 1:
                                    pend.pop(0)[1]()
                                    if len(pend) > 12 and pend[0][0] is not None:
                                        pend.pop(0)[1]()
                            elif p == NP - 1:
                                if gi >= 1:
                                    pend.pop(0)[1]()
                            elif qb >= 1 and (gi % 2 == 1 or len(pend) > (NQB - qb) * 4):
                                pend.pop(0)[1]()
                        if pend_late and ((qb == 2 and gi >= 1) or (qb == 3 and gi >= 5)):
                            pend_late.pop(0)()

                    # ---- evacuate y (unnormalized) + packed sums ----
                    qsl = slice(QB * qb, QB * (qb + 1))
                    nc.vector.tensor_copy(out=yT_sb[p][0:64, qsl], in_=y_ps[0][0:64, :])
                    ytmp = att.tile([64, 512], BF16, tag="ytmp", bufs=2)
                    nc.vector.tensor_copy(out=ytmp[:], in_=y_ps[1][0:64, :])
                    nc.sync.dma_start(out=yT_sb[p][64:128, qsl], in_=ytmp[:])
                    for h in range(2):
                        stg = att.tile([65, 512], F32, tag=f"sumstg{h}", bufs=2, name=f"stg{h}")
                        nc.vector.tensor_copy(out=stg[64:65, :], in_=y_ps[h][64:65, :])
                        u = 8 * p + 2 * qb + h
                        nc.sync.dma_start(out=sums_pack[4 * u:4 * (u + 1), :], in_=stg[64:65, :])
                    # per-qb normalize (recip over the pair's 32 rows is idempotent)
                    prows = slice(32 * p, 32 * (p + 1))
                    nc.vector.reciprocal(out=recip_pack[prows, :], in_=sums_pack[prows, :])
                    nc.vector.tensor_copy(out=recip_pbf[prows, :], in_=recip_pack[prows, :])
                    for h in range(2):
                        bse = 32 * p + 8 * qb + 4 * h
                        nc.sync.dma_start(
                            out=rdram[2 * p + h:2 * p + h + 1, qsl],
                            in_=recip_pbf[bse:bse + 4, :],
                        )
                    rb = att.tile([128, QB], BF16, tag="rb", bufs=2)
                    nc.sync.dma_start(out=rb[0:64, :], in_=rdram[2 * p:2 * p + 1, qsl].to_broadcast([64, QB]))
                    nc.sync.dma_start(out=rb[64:128, :], in_=rdram[2 * p + 1:2 * p + 2, qsl].to_broadcast([64, QB]))
                    nc.vector.tensor_mul(out=yT_sb[p][:, qsl], in0=yT_sb[p][:, qsl], in1=rb[:])

                    if p == NP - 1:
                        # quarter AllGather of this q block's y^T
                        nc.sync.dma_start(out=cc_in4[qb][:], in_=yT_sb[p][:, qsl])
                        nc.gpsimd.collective_compute(
                            "AllGather",
                            mybir.AluOpType.bypass,
                            replica_groups=GROUPS2,
                            ins=[cc_in4[qb][:].opt()],
                            outs=[cc_out4[qb][:].opt()],
                        )
                        if qb < NQB - 1:
                            def yg_q(qb=qb):
                                qs = slice(QB * qb, QB * (qb + 1))
                                nc.sync.dma_start(out=yg_sb[NP - 1][:, qs], in_=cc_out4[qb][0:128, :])
                                nc.sync.dma_start(out=yg_sb[2 * NP - 1][:, qs], in_=cc_out4[qb][128:256, :])
                            pend_late.append(yg_q)
                            for tt in range(4 * qb, 4 * qb + 4):
                                pend_late.append(lambda tt=tt: proj_full(tt))
                            if qb == 2:
                                # partials only need pairs 0-2: fine anywhere in qb3
                                for tt in range(12, 16):
                                    pend.append((None, lambda tt=tt: proj_partial(tt)))

                while pend:
                    pend.pop(0)[1]()
                while pend_late:
                    pend_late.pop(0)()
                if p < NP - 1:
                    nc.sync.dma_start(out=cc_in[p][:], in_=yT_sb[p][:])
                    nc.gpsimd.collective_compute(
                        "AllGather",
                        mybir.AluOpType.bypass,
                        replica_groups=GROUPS2,
                        ins=[cc_in[p][:].opt()],
                        outs=[cc_out[p][:].opt()],
                    )

            # ---- tail: last quarter arrives, finish tts 12..15 ----
            qs3 = slice(QB * 3, QB * 4)
            nc.sync.dma_start(out=yg_sb[NP - 1][:, qs3], in_=cc_out4[3][0:128, :])
            nc.scalar.dma_start(out=yg_sb[2 * NP - 1][:, qs3], in_=cc_out4[3][128:256, :])
            for tt in range(12, 16):
                o_ps2 = ps.tile([128, 512], F32, tag="qkv", bufs=2)
                for ri in range(2):
                    nc.tensor.matmul(
                        o_ps2[:],
                        yg_sb[ri * NP + NP - 1][:, 128 * tt:128 * (tt + 1)],
                        wp_sb[4 * ri + NP - 1][:],
                        start=(ri == 0), stop=(ri == 1),
                    )
                o_sb = att.tile([128, 512], F32, tag="osb", bufs=3)
                nc.vector.scalar_tensor_tensor(
                    out=o_sb[:], in0=o_ps2[:], scalar=1.0, in1=o_part[tt - 12][:],
                    op0=mybir.AluOpType.mult, op1=mybir.AluOpType.add,
                )
                eng = nc.sync if tt % 2 == 0 else nc.scalar
                eng.dma_start(out=out.ap()[128 * tt:128 * (tt + 1), :], in_=o_sb[:])

    nc.compile()
    _NC_CACHE["nc"] = nc
    return nc


def make_in_maps(x, w_qkv, w_proj):
    bf = ml_dtypes.bfloat16
    # 256-wide causal staircase masks for the two diagonal kv-tile offsets
    i = np.arange(128)[:, None]
    j = np.arange(256)[None, :]
    msk2 = np.concatenate(
        [(i <= j).astype(bf), (128 + i <= j).astype(bf)], axis=1
    )  # [128, 512]
    in_maps = []
    for core in range(8):
        beta, eta = core // 2, core % 2
        xT = np.ascontiguousarray(x[beta].T).astype(bf)
        wk = w_qkv[:, C + 512 * eta: C + 512 * (eta + 1)]
        wq = w_qkv[:, 512 * eta: 512 * (eta + 1)]
        wv = w_qkv[:, 2 * C + 512 * eta: 2 * C + 512 * (eta + 1)]
        wkqv = np.ascontiguousarray(
            np.concatenate([wk, wq, wv], axis=1)).astype(bf)
        wp = w_proj[:, 512 * eta: 512 * (eta + 1)].astype(bf)
        in_maps.append({"xT": xT, "wkqv": wkqv,
                        "wp": np.ascontiguousarray(wp), "msk2": msk2})
    return in_maps


def assemble(results):
    out = np.empty((B, T, C), np.float32)
    for core in range(8):
        beta, eta = core // 2, core % 2
        out[beta, :, 512 * eta: 512 * (eta + 1)] = results[core]["out"]
    return out


def kernel(x, w_qkv, w_proj):
    x = np.asarray(x, np.float32)
    w_qkv = np.asarray(w_qkv, np.float32)
    w_proj = np.asarray(w_proj, np.float32)
    nc = build_nc()
    in_maps = make_in_maps(x, w_qkv, w_proj)
    res = run_bass_kernel_spmd(nc, in_maps, core_ids=list(range(8)))
    return assemble(res.results)
